# revision 18
# baseline (speedup 1.0000x reference)
"""Trainium2 Bass kernel for nn_CascadeGradNetOURS (dense_mlp).

Math (reference):
    h = x @ W.T                       # (B, E), shared by all layers
    z = beta[0] * (h + b[0])
    for i in 0..6:
        z = beta[i+1]*(h + b[i+1]) + alpha[i]*relu(z)
    z = alpha[7] * relu(z)
    out = z @ W + bias_last           # (B, IN)

Device formulation (per core, batch-sharded 1024 rows, transposed layout
hT[e, b] so per-layer alpha/beta/bias become per-PARTITION scalars).

Raw-form cascade (no sign tricks needed -- signed per-partition scalars ride
in tensor_scalar multiplies, which hit the DVE 4x perf mode):
    htil_j = beta_j * h + beta_j*b_j        (j=0..7)   Act identity-act / Pool ts
    z_1    = htil_0
    u_i    = (z_i max 0) * alpha_i          (i=0..7)   DVE/Pool ts (max,mult)
    z_i+1  = htil_i+1 + u_i                 (i=0..6)   DVE tensor_tensor add
    z      = u_7  (written straight into z_sb for mm2)

Engine balance (measured rates, ns/col): DVE ts .53 / tt .60, Act act 1.21
(1.09 from PSUM), Pool ts 1.17. Assignment approximates the LP optimum:
all joins DVE; u: ~5.5/8 DVE, rest Pool; htil: ~4.5/8 Act, rest Pool.

mm2 (z @ W) runs in three PE stages around PSUM's 8 banks:
  B-W1: out half hf=1, ecs 0..15 burst right after mm1, stash to fp16 o_acc
  A:    out half hf=0, all 32 ecs, ec-major gated on z[ec] during cascade
  B-W2: out half hf=1, ecs 16..31 after cascade, combined with o_acc
"""

import os

os.environ.setdefault("MYCRO_LOCAL_CACHE", "1")

import numpy as np

import concourse.bacc as bacc
import concourse.bass as bass
import concourse.mybir as mybir
from concourse.tile import TileContext

N_CORES = 8
B, IN, E, L = 8192, 1024, 4096, 8
BC = B // N_CORES          # 1024 batch rows per core
NI = IN // 128             # 8 i-chunks
NE = E // 128              # 32 e-chunks
F16 = mybir.dt.float16
F32 = mybir.dt.float32
NCONST = 24                # cols: 0..7 beta_j, 8..15 Bb_j, 16..23 alpha_i

GROUP = 4                  # e-chunks emitted per cascade wave
SWEEPS = ((0, 16), (16, 26), (26, 32))   # mm2 contraction windows over ecs

# per-ec op->engine maps (k = layer index).  'A'=Act, 'D'=DVE, 'P'=Pool
# htil_0 is always Act (from PSUM); joins always DVE.
# Act executes in order, so during mm1 it must stay copy-only or PSUM stops
# draining at PE pace: early groups (cascaded while mm1 runs) use Pool htils;
# later groups (cascaded after mm1) lean on the then-idle Act.
EARLY = 12                  # ecs below this cascade while mm1 is running

def htil_engine(ec, j):
    if j == 0:
        return "A"          # from PSUM (fused with the h16 copy slot)
    if ec < EARLY:
        return "P"
    return "A" if j in (1, 2, 3, 4) else "P"

def u_engine(ec, i):
    if ec < EARLY:
        return "D"
    if i in (3, 5):
        return "P"
    if i == 6:
        return "P" if ec % 2 == 1 else "D"
    return "D"


_SEQ_ONLY = {
    "InstUnconditionalBranch",
    "InstCall",
    "InstISA",
}


def _legalize_waits(nc):
    """Datapath instructions carry exactly ONE semaphore wait slot in the
    64-byte ISA encoding (walrus errors on more). Engine sequencers execute
    their stream in order, so any extra waits can be hoisted onto single-wait
    NoOps inserted immediately before the capped instruction."""
    import bass_rust

    uid = 0
    for bb in nc.m.functions[0].blocks:
        insts = bb.instructions  # live list
        newlist = []
        for i in insts:
            cls = i.__class__.__name__
            si = i.sync_info
            if cls in _SEQ_ONLY or si is None or len(si.on_wait) <= 1:
                newlist.append(i)
                continue
            waits = list(si.on_wait)
            if cls == "InstDMACopy":
                dmaw = [w for w in waits if w.ant_name.startswith("DMA")]
                keep = dmaw[-1] if dmaw else waits[-1]
            else:
                keep = waits[-1]
            rest = [w for w in waits if w is not keep]
            for w in rest:
                uid += 1
                nop = mybir.InstNoOp(
                    name=f"waitnop-{uid}-{i.name}",
                    engine=i.engine,
                    bass_nofuse=True,
                )
                nop.sync_info = bass_rust.SyncInfo(on_wait=[w], on_update=[])
                newlist.append(nop)
            si.on_wait = [keep]
            newlist.append(i)
        if len(newlist) != len(insts):
            insts[:] = newlist


def build_nc() -> bass.Bass:
    nc = bacc.Bacc()
    AL = mybir.AluOpType
    AF = mybir.ActivationFunctionType

    xTd = nc.declare_dram_parameter("xT", [128, NI, BC], F16, isOutput=False)
    WTd = nc.declare_dram_parameter("WT", [128, NE, NI, 128], F16, isOutput=False)
    W2d = nc.declare_dram_parameter("W2", [128, NE, IN], F16, isOutput=False)
    Cd = nc.declare_dram_parameter("consts", [128, NE, NCONST], F32, isOutput=False)
    Bd = nc.declare_dram_parameter("blast", [128, NI], F32, isOutput=False)
    Od = nc.declare_dram_parameter("outT", [128, NI, BC], F32, isOutput=True)

    with TileContext(nc) as tc:
        with (
            tc.tile_pool(name="persist", bufs=1) as persist,
            tc.tile_pool(name="wtp", bufs=6) as wtp,
            tc.tile_pool(name="w2p", bufs=1) as w2p,
            tc.tile_pool(name="hp16", bufs=12) as hp16,
            tc.tile_pool(name="htp", bufs=6) as htp,
            tc.tile_pool(name="utp", bufs=6) as utp,
            tc.tile_pool(name="ztp", bufs=6) as ztp,
            tc.tile_pool(name="outp", bufs=2) as outp,
            tc.tile_pool(name="psum_h", bufs=3, space="PSUM") as psum_h,
            tc.tile_pool(name="psum_o", bufs=2, space="PSUM") as psum_o,
        ):
            consts_sb = persist.tile([128, NE, NCONST], F32)
            nc.sync.dma_start(out=consts_sb, in_=Cd[:, :, :])
            blast_sb = persist.tile([128, NI], F32)
            nc.sync.dma_start(out=blast_sb, in_=Bd[:, :])
            x_sb = persist.tile([128, NI, BC], F16)
            for i in range(NI):
                nc.sync.dma_start(out=x_sb[:, i, :], in_=xTd[:, i, :])
            z_sb = persist.tile([128, NE, BC], F16)
            o_acc = persist.tile([128, NI, BC], F16)    # mm2 partials

            def c_ap(ec, col):
                return consts_sb[:, ec, col : col + 1]

            # ---------------- Phase A: mm1 + cascade ----------------
            h16 = {}
            zcur = {}

            def emit_cascade(ecs):
                """Cascade layers for a set of ecs, layer-major."""
                for k in range(0, L):           # u_k, then join to z_{k+2}
                    for ec in ecs:
                        al = c_ap(ec, 16 + k)
                        if k == L - 1:
                            u = z_sb[:, ec, :]
                        else:
                            u = utp.tile(
                                [128, BC], F16, tag="u", name=f"u_{ec}_{k}"
                            )
                        eng = nc.vector if u_engine(ec, k) == "D" else nc.gpsimd
                        eng.tensor_scalar(u, zcur[ec], 0.0, al, AL.max, AL.mult)
                        if k == L - 1:
                            continue
                        j = k + 1
                        ht = htp.tile(
                            [128, BC], F16, tag="ht", name=f"ht_{ec}_{j}"
                        )
                        if htil_engine(ec, j) == "A":
                            nc.scalar.activation(
                                out=ht, in_=h16[ec], func=AF.Identity,
                                bias=c_ap(ec, 8 + j), scale=c_ap(ec, j),
                            )
                        else:
                            nc.gpsimd.tensor_scalar(
                                ht, h16[ec], c_ap(ec, j), c_ap(ec, 8 + j),
                                AL.mult, AL.add,
                            )
                        z2 = ztp.tile(
                            [128, BC], F16, tag="z", name=f"z_{ec}_{k+2}"
                        )
                        nc.vector.tensor_tensor(out=z2, in0=ht, in1=u, op=AL.add)
                        zcur[ec] = z2

            # mm1 + PSUM drains group-by-group, with cascade(g-2) interleaved.
            # Early groups use Pool htils so Act's stream stays copy-dominated
            # while mm1 runs; later groups' Act htils may delay late copies,
            # which only eats PE idle time (PE waits on z for mm2 anyway).
            LAG = 2
            w2_half = {}
            for g0 in range(0, NE, GROUP):
                if g0 == 2 * GROUP:
                    # W2 first half (sweep-0 ecs) needed by mm2 at ~t=115us;
                    # stream it in off the startup critical path.
                    w2_half[0] = w2p.tile(
                        [128, 16, IN], F16, tag="w2", name="w2_half0"
                    )
                    for gg in range(2):
                        nc.sync.dma_start(
                            out=w2_half[0][:, gg * 8 : (gg + 1) * 8, :],
                            in_=W2d[:, gg * 8 : (gg + 1) * 8, :],
                        )
                if g0 >= LAG * GROUP:
                    # cascade(g-LAG) BEFORE this group's PSUM drains: its Act
                    # htils must precede copies(g) in Act's in-order stream,
                    # or pool-slot waits form a cycle (z/h16 slots freed by
                    # cascade ops that would sit behind the copies).
                    lg = g0 - LAG * GROUP
                    emit_cascade(range(lg, lg + GROUP))
                ecs = range(g0, g0 + GROUP)
                h_ps = {}
                for ec in ecs:
                    wt = wtp.tile([128, NI, 128], F16, tag="wt")
                    nc.sync.dma_start(out=wt, in_=WTd[:, ec, :, :])
                    hp = psum_h.tile([128, BC], F32, tag="h")
                    for i in range(NI):
                        lhsT = wt[:, i, :]
                        for hf in range(2):
                            nc.tensor.matmul(
                                hp[:, hf * 512 : (hf + 1) * 512],
                                lhsT,
                                x_sb[:, i, hf * 512 : (hf + 1) * 512],
                                start=(i == 0),
                                stop=(i == NI - 1),
                            )
                    h_ps[ec] = hp

                # h16 copy + htil_0 (z_1) from PSUM on Act
                for ec in ecs:
                    t = hp16.tile([128, BC], F16, tag="h16", name=f"h16_{ec}")
                    nc.scalar.activation(
                        out=t, in_=h_ps[ec], func=AF.Identity,
                        bias=0.0, scale=1.0,
                    )
                    h16[ec] = t
                    z1 = ztp.tile([128, BC], F16, tag="z", name=f"z_{ec}_1")
                    nc.scalar.activation(
                        out=z1, in_=h_ps[ec], func=AF.Identity,
                        bias=c_ap(ec, 8), scale=c_ap(ec, 0),
                    )
                    zcur[ec] = z1
            for g0 in range(NE - LAG * GROUP, NE, GROUP):
                emit_cascade(range(g0, g0 + GROUP))

            # ---------------- Phase B: mm2 (3 contraction sweeps) ----------
            # 16 out-tiles t=(ic,hf) rotate through 2 PSUM banks per sweep;
            # sweep 0 stashes to fp16 o_acc (+bias), sweep 1 accumulates into
            # o_acc, sweep 2 produces the fp32 result and DMAs it out.
            for s, (e0, e1) in enumerate(SWEEPS):
                if s == 1:
                    # second half of W2 (reuses the 32KB slot; the DMA waits
                    # for sweep-0's last read of half 0 via pool dependency)
                    w2_half[1] = w2p.tile(
                        [128, 16, IN], F16, tag="w2", name="w2_half1"
                    )
                    for gg in range(2):
                        nc.sync.dma_start(
                            out=w2_half[1][:, gg * 8 : (gg + 1) * 8, :],
                            in_=W2d[:, 16 + gg * 8 : 16 + (gg + 1) * 8, :],
                        )
                for t in range(16):
                    ic, hf = t // 2, t % 2
                    ops = psum_o.tile(
                        [128, 512], F32, tag="o", name=f"mm2_{s}_{t}"
                    )
                    for ec in range(e0, e1):
                        w2t = w2_half[ec // 16]
                        nc.tensor.matmul(
                            ops,
                            w2t[:, ec % 16, ic * 128 : (ic + 1) * 128],
                            z_sb[:, ec, hf * 512 : (hf + 1) * 512],
                            start=(ec == e0),
                            stop=(ec == e1 - 1),
                        )
                    oa = o_acc[:, ic, hf * 512 : (hf + 1) * 512]
                    if s == 0:
                        nc.scalar.activation(
                            out=oa, in_=ops, func=AF.Identity,
                            bias=blast_sb[:, ic : ic + 1], scale=1.0,
                        )
                    elif s == 1:
                        nc.vector.tensor_tensor(
                            out=oa, in0=oa, in1=ops, op=AL.add
                        )
                    else:
                        osb = outp.tile([128, 512], F32, tag="osb")
                        nc.vector.tensor_tensor(
                            out=osb, in0=oa, in1=ops, op=AL.add
                        )
                        nc.scalar.dma_start(
                            out=Od[:, ic, hf * 512 : (hf + 1) * 512], in_=osb
                        )

    nc.compile()
    return nc


def _prep_inputs(x, W, biases, bias_last, alpha, beta):
    """Host-side shard/relayout/constant precompute. Returns per-core in_maps."""
    x = np.asarray(x, np.float32)
    W = np.asarray(W, np.float32)
    biases = np.asarray(biases, np.float32)
    bias_last = np.asarray(bias_last, np.float32)
    alpha = np.asarray(alpha, np.float32)
    beta = np.asarray(beta, np.float32)

    consts = np.zeros((E, NCONST), np.float32)
    for j in range(L):
        consts[:, j] = beta[j]
        consts[:, 8 + j] = beta[j] * biases[j]
        consts[:, 16 + j] = alpha[j]
    consts_t = np.ascontiguousarray(
        consts.reshape(NE, 128, NCONST).transpose(1, 0, 2)
    )

    WT_t = np.ascontiguousarray(
        W.T.reshape(NI, 128, NE, 128).transpose(1, 2, 0, 3).astype(np.float16)
    )
    W2_t = np.ascontiguousarray(
        W.reshape(NE, 128, IN).transpose(1, 0, 2).astype(np.float16)
    )
    blast_t = np.ascontiguousarray(bias_last.reshape(NI, 128).T)

    in_maps = []
    for c in range(N_CORES):
        xc = x[c * BC : (c + 1) * BC]           # (BC, IN)
        xT = np.ascontiguousarray(
            xc.T.reshape(NI, 128, BC).transpose(1, 0, 2).astype(np.float16)
        )
        in_maps.append(
            {
                "xT": xT,
                "WT": WT_t,
                "W2": W2_t,
                "consts": consts_t,
                "blast": blast_t,
            }
        )
    return in_maps


_NC_CACHE = None


def _install_ntff_hook():
    """The agent image's antenv lacks axon_hooks; rebuild it from the boot
    helper so run_bass_kernel_spmd(trace=True) can capture NTFF profiles."""
    import sys
    import types

    if "antenv.axon_hooks" in sys.modules:
        return
    try:
        from trn_agent_boot.trn_boot import _ntff_profile_via_ctypes

        hook = _ntff_profile_via_ctypes("/opt/axon/libaxon_pjrt.so")
    except Exception:
        hook = None
    m = types.ModuleType("antenv.axon_hooks")
    m.get_axon_ntff_profile_hook = lambda: hook
    m.set_axon_ntff_profile_hook = lambda h: None
    sys.modules["antenv.axon_hooks"] = m


def run(inputs: dict, trace: bool = False):
    """Returns (out, BassKernelResults)."""
    global _NC_CACHE
    from concourse.bass_utils import run_bass_kernel_spmd

    if trace:
        _install_ntff_hook()

    if _NC_CACHE is None:
        _NC_CACHE = build_nc()
    nc = _NC_CACHE
    in_maps = _prep_inputs(**inputs)
    res = run_bass_kernel_spmd(nc, in_maps, list(range(N_CORES)), trace=trace)
    out = np.empty((B, IN), np.float32)
    for c in range(N_CORES):
        oc = np.asarray(res.results[c]["outT"])          # (128, NI, BC)
        out_core = oc.transpose(1, 0, 2).reshape(IN, BC) # (IN, BC) = outT
        out[c * BC : (c + 1) * BC] = out_core.T
    return out, res


def kernel(x, W, biases, bias_last, alpha, beta) -> np.ndarray:
    out, _ = run(
        dict(x=x, W=W, biases=biases, bias_last=bias_last, alpha=alpha, beta=beta)
    )
    return out
